# revision 1
# baseline (speedup 1.0000x reference)
"""Trainium2 Bass kernel for nn_DQNConv (conv stack -> linear -> legal-move
masked softmax), data-parallel over 8 NeuronCores.

Self-contained: takes FULL inputs as numpy arrays, shards batch across the 8
cores, runs one SPMD Bass program, returns the FULL [16384, 4096] float32
output.

Algorithm per core (2048 rows):
  - The three VALID 3x3 convs are expressed as dense matmuls with features on
    the SBUF partition dim and batch on the free dim (7x7x1 -> 800 -> 576 ->
    64), fused bias+relu on the PSUM->SBUF evacuation.
  - logits tile [128 rows, 4096] = feat_tile.T @ Wl.T via TensorE (f32r).
  - e = exp(logits) on ScalarE straight out of PSUM (logits are in [-3, 3]).
  - legal-move mask is scattered per-row by GPSIMD local_scatter (writes 1.0
    at each index; duplicate indices write the same value, so last-write-wins
    is exact; chunks of <=2046 columns due to the op's num_elems limit).
  - u = e * mask with a fused row-sum Z (VectorE scalar_tensor_tensor with
    accum_out), then out = u * (1/Z) and a straight HWDGE DMA to HBM.
    Illegal entries are exactly 0.0, matching the reference (whose
    exp(-1000-ish) terms underflow to 0).
"""

import sys
import os

for _p in ("/opt/trn_rl_repo", "/root/.axon_site/_ro/trn_rl_repo"):
    if os.path.isdir(_p) and _p not in sys.path:
        sys.path.append(_p)

import numpy as np

import concourse.bass as bass
import concourse.bacc as bacc
import concourse.mybir as mybir
import concourse.tile as tile
from concourse.bass_utils import run_bass_kernel_spmd

B, HW, OUT, K = 16384, 7, 4096, 64
NCORES = 8
BC = B // NCORES           # 2048 rows per core
NRT = BC // 128            # 16 row-tiles per core
NCHUNK = 8                 # conv batch chunks per core (CW=256 keeps f32r full-rate)
CW = BC // NCHUNK          # 512 columns per conv chunk
F0, F1, F2, F3 = 49, 800, 576, 64
SCATTER_CHUNKS = [(0, 2046), (2046, 2046), (4092, 4)]

dt = mybir.dt
AT = mybir.AluOpType
ACTF = mybir.ActivationFunctionType
F32R = dt.float32r

# dtype of the exp/mask/u tiles; bf16 halves DVE cost, fp32 is more precise
E_DT = dt.bfloat16 if os.environ.get("KERNEL_E_DT", "bf16") == "bf16" else dt.float32


def _ptiles(n):
    """Split a feature count into partition tiles of <=128."""
    out = []
    base = 0
    while base < n:
        out.append((base, min(128, n - base)))
        base += 128
    return out


def _build(reps=1, fori=0):
    nc = bacc.Bacc("TRN2", target_bir_lowering=False, debug=False)

    xT = nc.dram_tensor("xT", [F0, BC], F32R, kind="ExternalInput")
    m1 = nc.dram_tensor("m1", [F0, F1], F32R, kind="ExternalInput")
    # m2/m3 arrive pre-packed into partition tiles (one DMA each; the HWDGE
    # issues input DMAs strictly FIFO, so fewer/earlier loads shorten the
    # prologue before the first conv evacuation can run)
    m2 = nc.dram_tensor("m2", [128, 7 * F2], F32R, kind="ExternalInput")
    m3 = nc.dram_tensor("m3", [128, 5 * F3], F32R, kind="ExternalInput")
    wlT = nc.dram_tensor("wlT", [F3 + 1, OUT], F32R, kind="ExternalInput")
    bad = nc.dram_tensor("ball", [128, 13], dt.float32, kind="ExternalInput")
    pmi = nc.dram_tensor("pmi", [BC, 3 * K], dt.int16, kind="ExternalInput")
    outd = nc.dram_tensor("out", [BC, OUT], dt.float32, kind="ExternalOutput")

    t1 = _ptiles(F1)   # 7 tiles: 6x128 + 32
    t2 = _ptiles(F2)   # 5 tiles: 4x128 + 64
    KF = F3 + 1        # 65 = features + homogeneous 1-row for bl

    with tile.TileContext(nc) as tc:
        with (
            tc.tile_pool(name="w", bufs=1) as wp,
            tc.tile_pool(name="h", bufs=2) as hp,
            tc.tile_pool(name="b", bufs=3) as bp,
            tc.tile_pool(name="ps", bufs=4, space="PSUM") as pp,
        ):
            # ---- static loads -------------------------------------------------
            xT_sb = wp.tile([F0, BC], F32R, tag="xT")
            nc.sync.dma_start(out=xT_sb[:], in_=xT.ap())
            m1_sb = wp.tile([F0, F1], F32R, tag="m1")
            nc.sync.dma_start(out=m1_sb[:], in_=m1.ap())
            # one small DMA carries all 13 per-tile bias columns; the conv
            # evacuations depend on it, so it must land early
            ball_sb = wp.tile([128, 13], dt.float32, tag="ball")
            nc.sync.dma_start(out=ball_sb[:], in_=bad.ap())
            b_tiles = {}
            _col = 0
            for name, tl in (("b1", t1), ("b2", t2), ("b3", _ptiles(F3))):
                for i, (kb, kn) in enumerate(tl):
                    b_tiles[(name, i)] = ball_sb[:kn, _col:_col + 1]
                    _col += 1
            m2_all = wp.tile([128, 7 * F2], F32R, tag="m2a")
            nc.sync.dma_start(out=m2_all[:], in_=m2.ap())
            m2_sb = [m2_all[:kn, i * F2:(i + 1) * F2] for i, (kb, kn) in enumerate(t1)]
            m3_all = wp.tile([128, 5 * F3], F32R, tag="m3a")
            nc.sync.dma_start(out=m3_all[:], in_=m3.ap())
            m3_sb = [m3_all[:kn, i * F3:(i + 1) * F3] for i, (kb, kn) in enumerate(t2)]
            wl_sb = wp.tile([KF, OUT], F32R, tag="wl")
            nc.sync.dma_start(out=wl_sb[:], in_=wlT.ap())
            # all 16 row-tiles of scatter indices in one load
            ix_sb = wp.tile([128, NRT, 3 * K], dt.int16, tag="ix")
            nc.sync.dma_start(
                out=ix_sb[:],
                in_=pmi.ap().rearrange("(t p) j -> p t j", p=128),
            )
            ones_sb = wp.tile([128, K], E_DT, tag="ones")
            nc.vector.memset(ones_sb[:], 1.0)

            # ---- per-chunk conv + per-row-tile softmax ------------------------
            # fori>0 wraps the body in a hardware loop (timing-only path)
            import contextlib
            _loop = tc.For_i(0, fori, 1) if fori > 0 else contextlib.nullcontext()
            with _loop:
             for _rep in range(reps):
              for c in range(NCHUNK):
                cs = slice(c * CW, (c + 1) * CW)

                # L1: [49 x 800] -> h1 (relu(x@M1 + b1))
                h1 = []
                for i, (kb, kn) in enumerate(t1):
                    ps = pp.tile([kn, CW], dt.float32, tag="ps")
                    nc.tensor.matmul(
                        ps[:],
                        m1_sb[:, kb:kb + kn],
                        xT_sb[:, cs],
                        start=True, stop=True,
                    )
                    h = hp.tile([kn, CW], F32R, tag=f"h1_{i}")
                    nc.scalar.activation(h[:], ps[:], ACTF.Relu,
                                         bias=b_tiles[("b1", i)])
                    h1.append(h)

                # L2: [800 x 576]
                h2 = []
                for i, (mb, mn) in enumerate(t2):
                    ps = pp.tile([mn, CW], dt.float32, tag="ps")
                    for kt, (kb, kn) in enumerate(t1):
                        nc.tensor.matmul(
                            ps[:],
                            m2_sb[kt][:, mb:mb + mn],
                            h1[kt][:],
                            start=(kt == 0), stop=(kt == len(t1) - 1),
                        )
                    h = hp.tile([mn, CW], F32R, tag=f"h2_{i}")
                    nc.scalar.activation(h[:], ps[:], ACTF.Relu,
                                         bias=b_tiles[("b2", i)])
                    h2.append(h)

                # L3: [576 x 64] -> feat chunk [65, CW] (row 64 = ones)
                ps3 = pp.tile([F3, CW], dt.float32, tag="ps")
                for kt, (kb, kn) in enumerate(t2):
                    nc.tensor.matmul(
                        ps3[:],
                        m3_sb[kt],
                        h2[kt][:],
                        start=(kt == 0), stop=(kt == len(t2) - 1),
                    )
                feat = hp.tile([KF, CW], F32R, tag="feat")
                nc.scalar.activation(feat[:F3, :], ps3[:], ACTF.Relu,
                                     bias=b_tiles[("b3", 0)])
                nc.vector.memset(feat[F3:KF, :].bitcast(dt.float32), 1.0)

                # ---- phase B: 4 row-tiles of this chunk -----------------------
                for r in range(CW // 128):
                    rt = c * (CW // 128) + r
                    lhsT = feat[:, r * 128:(r + 1) * 128]

                    # logits in 4 quarter-width PSUM tiles (2 banks each, 4
                    # pool slots) so the next chunk's conv matmuls can grab a
                    # slot while this tile's exp passes still hold others
                    e = bp.tile([128, OUT], E_DT, tag="e")
                    for q in range(4):
                        psl = pp.tile([128, OUT // 4], dt.float32, tag="ps")
                        for nb in range(2):
                            ns = slice(q * 1024 + nb * 512,
                                       q * 1024 + (nb + 1) * 512)
                            nc.tensor.matmul(
                                psl[:, nb * 512:(nb + 1) * 512],
                                lhsT,
                                wl_sb[:, ns],
                                start=True, stop=True,
                            )
                        nc.scalar.activation(
                            e[:, q * 1024:(q + 1) * 1024], psl[:], ACTF.Exp)

                    msk = bp.tile([128, OUT], E_DT, tag="msk")
                    for ci, (base, size) in enumerate(SCATTER_CHUNKS):
                        nc.gpsimd.local_scatter(
                            out_ap=msk[:, base:base + size],
                            data_ap=ones_sb[:],
                            idxs_ap=ix_sb[:, rt, ci * K:(ci + 1) * K],
                            channels=128, num_elems=size, num_idxs=K)

                    # u = e*mask on the 2x bf16 tensor_tensor path, then an
                    # identity tensor_scalar pass accumulates Z per row (the
                    # fused scalar_tensor_tensor accum runs at 1x; this split
                    # is faster on the DVE).
                    u = e
                    z = bp.tile([128, 1], dt.float32, tag="z")
                    nc.vector.tensor_mul(u[:], e[:], msk[:])
                    nc.vector.tensor_scalar(
                        out=u[:], in0=u[:], scalar1=1.0, scalar2=0.0,
                        op0=AT.mult, op1=AT.add, accum_out=z[:])
                    rz = bp.tile([128, 1], dt.float32, tag="rz")
                    nc.vector.reciprocal(rz[:], z[:])
                    o = bp.tile([128, OUT], dt.float32, tag="o")
                    nc.vector.tensor_scalar(
                        out=o[:], in0=u[:], scalar1=rz[:], scalar2=None,
                        op0=AT.mult)
                    nc.sync.dma_start(
                        out=outd.ap()[rt * 128:(rt + 1) * 128, :], in_=o[:])

    nc.compile()
    return nc


_CACHE = {}


def _get_nc(reps=1, fori=0):
    key = ("nc", reps, fori)
    if key not in _CACHE:
        _CACHE[key] = _build(reps, fori)
    return _CACHE[key]


def _conv_mats(W1, W2, W3):
    """Dense [in_feat, out_feat] matrices for the three VALID 3x3 convs with
    channel-major (c, y, x) feature flattening on both sides."""
    M1 = np.zeros((F0, F1), np.float32)
    for ky in range(3):
        for kx in range(3):
            for oy in range(5):
                for ox in range(5):
                    # row = input pixel, col = (oc, oy, ox)
                    M1[(oy + ky) * 7 + (ox + kx),
                       np.arange(32) * 25 + oy * 5 + ox] = W1[:, 0, ky, kx]
    M2 = np.zeros((F1, F2), np.float32)
    ic = np.arange(32)
    for ky in range(3):
        for kx in range(3):
            for oy in range(3):
                for ox in range(3):
                    rows = ic * 25 + (oy + ky) * 5 + (ox + kx)      # [32]
                    cols = np.arange(64) * 9 + oy * 3 + ox           # [64]
                    M2[np.ix_(rows, cols)] = W2[:, :, ky, kx].T      # [32,64]
    M3 = W3.transpose(1, 2, 3, 0).reshape(F2, F3).astype(np.float32)
    return M1, M2, M3


def kernel(**inputs):
    x = np.ascontiguousarray(np.asarray(inputs["x"], dtype=np.float32)).reshape(B, F0)
    pm = np.asarray(inputs["possible_moves"]).astype(np.int32, copy=False)
    W1 = np.asarray(inputs["W1"], dtype=np.float32)
    b1 = np.asarray(inputs["b1"], dtype=np.float32)
    W2 = np.asarray(inputs["W2"], dtype=np.float32)
    b2 = np.asarray(inputs["b2"], dtype=np.float32)
    W3 = np.asarray(inputs["W3"], dtype=np.float32)
    b3 = np.asarray(inputs["b3"], dtype=np.float32)
    Wl = np.asarray(inputs["Wl"], dtype=np.float32)
    bl = np.asarray(inputs["bl"], dtype=np.float32)

    M1, M2, M3 = _conv_mats(W1, W2, W3)
    WlT = np.concatenate([Wl.T.astype(np.float32), bl[None, :]], axis=0)
    # pack conv matrices into [128, n_tiles*width] partition tiles and all 13
    # per-tile bias columns into one [128, 13] array (single DMA each)
    M2p = np.zeros((128, 7 * F2), np.float32)
    for i, kb in enumerate(range(0, F1, 128)):
        kn = min(128, F1 - kb)
        M2p[:kn, i * F2:(i + 1) * F2] = M2[kb:kb + kn]
    M3p = np.zeros((128, 5 * F3), np.float32)
    for i, kb in enumerate(range(0, F2, 128)):
        kn = min(128, F2 - kb)
        M3p[:kn, i * F3:(i + 1) * F3] = M3[kb:kb + kn]
    b1v = np.repeat(b1, 25).astype(np.float32)
    b2v = np.repeat(b2, 9).astype(np.float32)
    ball = np.zeros((128, 13), np.float32)
    _col = 0
    for vec, width in ((b1v, 128), (b2v, 128), (b3.astype(np.float32), 128)):
        for kb in range(0, len(vec), 128):
            kn = min(128, len(vec) - kb)
            ball[:kn, _col] = vec[kb:kb + kn]
            _col += 1
    assert _col == 13

    # per-row scatter indices, chunked to local_scatter's num_elems limit
    pmi = np.empty((B, 3, K), np.int16)
    for ci, (base, size) in enumerate(SCATTER_CHUNKS):
        inr = (pm >= base) & (pm < base + size)
        pmi[:, ci, :] = np.where(inr, pm - base, -1).astype(np.int16)
    pmi = pmi.reshape(B, 3 * K)

    xTall = np.ascontiguousarray(x.T)   # [49, B]

    nc = _get_nc()
    in_maps = []
    for c in range(NCORES):
        sl = slice(c * BC, (c + 1) * BC)
        in_maps.append({
            "xT": np.ascontiguousarray(xTall[:, sl]),
            "m1": M1, "m2": M2p, "m3": M3p, "wlT": WlT,
            "ball": ball,
            "pmi": np.ascontiguousarray(pmi[sl]),
        })

    trace = bool(int(os.environ.get("KERNEL_TRACE", "0")))
    res = run_bass_kernel_spmd(nc, in_maps, list(range(NCORES)), trace=trace)
    _CACHE["last_results"] = res
    out = np.concatenate([res.results[i]["out"] for i in range(NCORES)], axis=0)
    return out



# revision 16
# speedup vs baseline: 2.7589x; 2.7589x over previous
"""Trainium2 Bass kernel for nn_DQNConv (conv stack -> linear -> legal-move
masked softmax), data-parallel over 8 NeuronCores.

Self-contained: takes FULL inputs as numpy arrays, shards batch across the 8
cores, runs one SPMD Bass program, returns the FULL [16384, 4096] float32
output.

v2 (vs the 167.8us baseline): the engine-busy profile of the baseline was
Act 109us / DMA 108us / Pool 97us / DVE 91us / PE 69us -- nearly balanced, so
the makespan was driven by the f32 output DMA and the Activation engine.
Changes:
  - output tensor is bf16 (DMA out 101us -> 47us); the host upcasts to f32.
    Illegal entries stay exactly 0.0 and bf16 rounding (<0.4%) is far inside
    the 2e-2 gate.
  - conv chunks are CW=512 (half the relu-evacuation instructions), and the
    13 per-chunk evacuations are split Act(8)/DVE(5) to rebalance those two
    engines under the GPSIMD scatter (the new bottleneck at ~96us).
  - the final scale pass writes bf16 (DVE 4x mode, 1.1us vs 2.1us per tile).
  - logits use one 1024-wide matmul per PSUM quarter-tile.
  - feat's homogeneous ones-row is written once in the prologue (the per-chunk
    evacuation only touches rows 0..63, so it survives buffer rotation).
  - scatter indices are packed on the host to [128, t, j] so their load is 2
    large-descriptor DMAs and lands right after x, letting the Pool engine
    start mask scatters ~immediately.

Algorithm per core (2048 rows):
  - The three VALID 3x3 convs are dense matmuls with features on the SBUF
    partition dim and batch on the free dim (49 -> 800 -> 576 -> 64), fused
    bias+relu on the PSUM->SBUF evacuation (split across ScalarE/VectorE).
  - logits tile [128 rows, 4096] = feat_tile.T @ Wl.T via TensorE (f32r).
  - e = exp(logits) on ScalarE straight out of PSUM (logits are in [-3, 3]).
  - legal-move mask is scattered per-row by GPSIMD local_scatter (writes 1.0
    at each index; duplicate indices write the same value, so last-write-wins
    is exact; chunks of <=2046 columns due to the op's num_elems limit).
  - u = e * mask (VectorE tensor_tensor, bf16 2x), Z via an identity
    tensor_scalar pass with accum_out (bf16 4x), out = u * (1/Z) in bf16 and
    a straight HWDGE DMA to HBM.
"""

import sys
import os

for _p in ("/opt/trn_rl_repo", "/root/.axon_site/_ro/trn_rl_repo"):
    if os.path.isdir(_p) and _p not in sys.path:
        sys.path.append(_p)

import numpy as np

import concourse.bass as bass
import concourse.bacc as bacc
import concourse.mybir as mybir
import concourse.tile as tile
from concourse.bass_utils import run_bass_kernel_spmd

B, HW, OUT, K = 16384, 7, 4096, 64
NCORES = 8
BC = B // NCORES           # 2048 rows per core
NRT = BC // 128            # 16 row-tiles per core
NCHUNK = 4                 # conv batch chunks per core
CW = BC // NCHUNK          # 512 columns per conv chunk
RPC = CW // 128            # 4 row-tiles per chunk
F0, F1, F2, F3 = 49, 800, 576, 64
# GPSIMD local_scatter covers columns 0..4091 (num_elems <= 2046); the last 4
# columns' mask is precomputed on the host and copied in by VectorE.
SCATTER_CHUNKS = [(0, 2046), (2046, 2046)]
MK4_BASE = 4092

dt = mybir.dt
AT = mybir.AluOpType
ACTF = mybir.ActivationFunctionType
F32R = dt.float32r
E_DT = dt.bfloat16


def _ptiles(n):
    """Split a feature count into partition tiles of <=128."""
    out = []
    base = 0
    while base < n:
        out.append((base, min(128, n - base)))
        base += 128
    return out


def _build(reps=1, fori=0):
    nc = bacc.Bacc("TRN2", target_bir_lowering=False, debug=False)

    xT = nc.dram_tensor("xT", [F0, BC], F32R, kind="ExternalInput")
    m1 = nc.dram_tensor("m1", [F0, F1], F32R, kind="ExternalInput")
    # m2/m3 arrive pre-packed into partition tiles (one DMA each)
    m2 = nc.dram_tensor("m2", [128, 7 * F2], F32R, kind="ExternalInput")
    m3 = nc.dram_tensor("m3", [128, 5 * F3], F32R, kind="ExternalInput")
    wlT = nc.dram_tensor("wlT", [F3 + 1, OUT], F32R, kind="ExternalInput")
    bad = nc.dram_tensor("ball", [128, 13], dt.float32, kind="ExternalInput")
    # scatter indices pre-packed on host to [p, t, j] layout
    pmi = nc.dram_tensor("pmi", [128, NRT * 2 * K], dt.int16, kind="ExternalInput")
    # host-precomputed mask for the last 4 columns
    mk4 = nc.dram_tensor("mk4", [128, NRT * 4], E_DT, kind="ExternalInput")
    outd = nc.dram_tensor("out", [BC, OUT], E_DT, kind="ExternalOutput")

    t1 = _ptiles(F1)   # 7 tiles: 6x128 + 32
    t2 = _ptiles(F2)   # 5 tiles: 4x128 + 64
    KF = F3 + 1        # 65 = features + homogeneous 1-row for bl

    with tile.TileContext(nc) as tc:
        with (
            tc.tile_pool(name="w", bufs=1) as wp,
            tc.tile_pool(name="h", bufs=2) as hp,
            tc.tile_pool(name="b", bufs=3) as bp,
            tc.tile_pool(name="m", bufs=5) as mp,
            tc.tile_pool(name="psc", bufs=3, space="PSUM") as pcp,
            tc.tile_pool(name="psl", bufs=2, space="PSUM") as plp,
        ):
            # ---- static loads -------------------------------------------------
            # scatter indices first: the Pool engine (the bottleneck) needs
            # only these, so mask scatters start during the conv prologue and
            # run ahead through the 5 msk buffers
            ix_sb = wp.tile([128, NRT, 2 * K], dt.int16, tag="ix")
            nc.sync.dma_start(
                out=ix_sb[:],
                in_=pmi.ap().rearrange("p (t j) -> p t j", t=NRT),
            )
            # one small DMA carries all 13 per-tile bias columns; the conv
            # evacuations depend on it, so it must land early
            ball_sb = wp.tile([128, 13], dt.float32, tag="ball")
            nc.sync.dma_start(out=ball_sb[:], in_=bad.ap())
            b_tiles = {}
            _col = 0
            for name, tl in (("b1", t1), ("b2", t2), ("b3", _ptiles(F3))):
                for i, (kb, kn) in enumerate(tl):
                    b_tiles[(name, i)] = ball_sb[:kn, _col:_col + 1]
                    _col += 1
            m1_sb = wp.tile([F0, F1], F32R, tag="m1")
            nc.sync.dma_start(out=m1_sb[:], in_=m1.ap())
            mk4_sb = wp.tile([128, NRT, 4], E_DT, tag="mk4")
            nc.sync.dma_start(
                out=mk4_sb[:],
                in_=mk4.ap().rearrange("p (t j) -> p t j", t=NRT),
            )
            xT_sb = wp.tile([F0, BC], F32R, tag="xT")
            nc.sync.dma_start(out=xT_sb[:], in_=xT.ap())
            m2_all = wp.tile([128, 7 * F2], F32R, tag="m2a")
            nc.sync.dma_start(out=m2_all[:], in_=m2.ap())
            m2_sb = [m2_all[:kn, i * F2:(i + 1) * F2] for i, (kb, kn) in enumerate(t1)]
            m3_all = wp.tile([128, 5 * F3], F32R, tag="m3a")
            nc.sync.dma_start(out=m3_all[:], in_=m3.ap())
            m3_sb = [m3_all[:kn, i * F3:(i + 1) * F3] for i, (kb, kn) in enumerate(t2)]
            wl_sb = wp.tile([KF, OUT], F32R, tag="wl")
            nc.sync.dma_start(out=wl_sb[:], in_=wlT.ap())
            ones_sb = wp.tile([128, K], E_DT, tag="ones")
            nc.vector.memset(ones_sb[:], 1.0)

            # evacuation engine split: ScalarE gets 8 per chunk, VectorE 5
            def evac(engine, h_ap, ps_ap, bias_ap):
                if engine == "act":
                    nc.scalar.activation(h_ap, ps_ap, ACTF.Relu, bias=bias_ap)
                else:
                    nc.vector.tensor_scalar(
                        out=h_ap, in0=ps_ap,
                        scalar1=bias_ap, scalar2=0.0,
                        op0=AT.add, op1=AT.max)

            EV1 = ["act", "dve", "act", "dve", "act", "dve", "act"]
            EV2 = ["act", "dve", "act", "dve", "act"]

            # ---- phase B emitter: one row-tile of masked softmax --------------
            def phase_b(feat_t, rt, r):
                lhsT = feat_t[:, r * 128:(r + 1) * 128]

                e = bp.tile([128, OUT], E_DT, tag="e")
                for q in range(4):
                    psl = plp.tile([128, OUT // 4], dt.float32, tag="ps")
                    for nb in range(2):
                        ns = slice(q * 1024 + nb * 512,
                                   q * 1024 + (nb + 1) * 512)
                        nc.tensor.matmul(
                            psl[:, nb * 512:(nb + 1) * 512],
                            lhsT,
                            wl_sb[:, ns],
                            start=True, stop=True,
                        )
                    nc.scalar.activation(
                        e[:, q * 1024:(q + 1) * 1024], psl[:], ACTF.Exp)

                msk = mp.tile([128, OUT], E_DT, tag="msk")
                for ci, (base, size) in enumerate(SCATTER_CHUNKS):
                    nc.gpsimd.local_scatter(
                        out_ap=msk[:, base:base + size],
                        data_ap=ones_sb[:],
                        idxs_ap=ix_sb[:, rt, ci * K:(ci + 1) * K],
                        channels=128, num_elems=size, num_idxs=K)
                nc.vector.tensor_scalar(
                    out=msk[:, MK4_BASE:OUT], in0=mk4_sb[:, rt, :],
                    scalar1=1.0, scalar2=None, op0=AT.mult)

                # u = e*mask on the 2x bf16 tensor_tensor path, then an
                # identity tensor_scalar pass accumulates Z per row (4x)
                u = e
                z = bp.tile([128, 1], dt.float32, tag="z")
                nc.vector.tensor_mul(u[:], e[:], msk[:])
                nc.vector.tensor_scalar(
                    out=u[:], in0=u[:], scalar1=1.0, scalar2=0.0,
                    op0=AT.mult, op1=AT.add, accum_out=z[:])
                rz = bp.tile([128, 1], dt.float32, tag="rz")
                nc.vector.reciprocal(rz[:], z[:])
                o = bp.tile([128, OUT], E_DT, tag="o")
                nc.vector.tensor_scalar(
                    out=o[:], in0=u[:], scalar1=rz[:], scalar2=None,
                    op0=AT.mult)
                nc.sync.dma_start(
                    out=outd.ap()[rt * 128:(rt + 1) * 128, :], in_=o[:])

            # ---- per-chunk conv, software-pipelined one chunk ahead of the
            # masked softmax (phase B of chunk c-1 is interleaved into chunk
            # c's conv emission so the in-order PE/Act queues stay fed)
            # fori>0 wraps the body in a hardware loop (timing-only path)
            import contextlib
            _loop = tc.For_i(0, fori, 1) if fori > 0 else contextlib.nullcontext()
            with _loop:
             for _rep in range(reps):
              prev_feat = None
              for c in range(NCHUNK):
                cs = slice(c * CW, (c + 1) * CW)

                # L1: [49 x 800] -> h1 (relu(x@M1 + b1))
                h1 = []
                for i, (kb, kn) in enumerate(t1):
                    ps = pcp.tile([kn, CW], dt.float32, tag="ps")
                    nc.tensor.matmul(
                        ps[:],
                        m1_sb[:, kb:kb + kn],
                        xT_sb[:, cs],
                        start=True, stop=True,
                    )
                    h = hp.tile([kn, CW], F32R, tag=f"h1_{i}")
                    evac(EV1[i], h[:], ps[:], b_tiles[("b1", i)])
                    h1.append(h)

                if prev_feat is not None:
                    phase_b(prev_feat, (c - 1) * RPC + 0, 0)
                    phase_b(prev_feat, (c - 1) * RPC + 1, 1)

                # L2: [800 x 576]
                h2 = []
                for i, (mb, mn) in enumerate(t2):
                    ps = pcp.tile([mn, CW], dt.float32, tag="ps")
                    for kt, (kb, kn) in enumerate(t1):
                        nc.tensor.matmul(
                            ps[:],
                            m2_sb[kt][:, mb:mb + mn],
                            h1[kt][:],
                            start=(kt == 0), stop=(kt == len(t1) - 1),
                        )
                    h = hp.tile([mn, CW], F32R, tag=f"h2_{i}")
                    evac(EV2[i], h[:], ps[:], b_tiles[("b2", i)])
                    h2.append(h)

                if prev_feat is not None:
                    phase_b(prev_feat, (c - 1) * RPC + 2, 2)

                # L3: [576 x 64] -> feat chunk [65, CW] (row 64 = ones,
                # pre-written in the prologue)
                ps3 = pcp.tile([F3, CW], dt.float32, tag="ps")
                for kt, (kb, kn) in enumerate(t2):
                    nc.tensor.matmul(
                        ps3[:],
                        m3_sb[kt],
                        h2[kt][:],
                        start=(kt == 0), stop=(kt == len(t2) - 1),
                    )
                feat = hp.tile([KF, CW], F32R, tag="feat")
                nc.scalar.activation(feat[:F3, :], ps3[:], ACTF.Relu,
                                     bias=b_tiles[("b3", 0)])
                nc.vector.memset(feat[F3:KF, :].bitcast(dt.float32), 1.0)

                if prev_feat is not None:
                    phase_b(prev_feat, (c - 1) * RPC + 3, 3)
                prev_feat = feat

              # drain: phase B of the last chunk
              for r in range(RPC):
                phase_b(prev_feat, (NCHUNK - 1) * RPC + r, r)

    nc.compile()
    return nc


_CACHE = {}


def _get_nc(reps=1, fori=0):
    key = ("nc", reps, fori)
    if key not in _CACHE:
        _CACHE[key] = _build(reps, fori)
    return _CACHE[key]


def _conv_mats(W1, W2, W3):
    """Dense [in_feat, out_feat] matrices for the three VALID 3x3 convs with
    channel-major (c, y, x) feature flattening on both sides."""
    M1 = np.zeros((F0, F1), np.float32)
    for ky in range(3):
        for kx in range(3):
            for oy in range(5):
                for ox in range(5):
                    # row = input pixel, col = (oc, oy, ox)
                    M1[(oy + ky) * 7 + (ox + kx),
                       np.arange(32) * 25 + oy * 5 + ox] = W1[:, 0, ky, kx]
    M2 = np.zeros((F1, F2), np.float32)
    ic = np.arange(32)
    for ky in range(3):
        for kx in range(3):
            for oy in range(3):
                for ox in range(3):
                    rows = ic * 25 + (oy + ky) * 5 + (ox + kx)      # [32]
                    cols = np.arange(64) * 9 + oy * 3 + ox           # [64]
                    M2[np.ix_(rows, cols)] = W2[:, :, ky, kx].T      # [32,64]
    M3 = W3.transpose(1, 2, 3, 0).reshape(F2, F3).astype(np.float32)
    return M1, M2, M3


def kernel(**inputs):
    x = np.ascontiguousarray(np.asarray(inputs["x"], dtype=np.float32)).reshape(B, F0)
    pm = np.asarray(inputs["possible_moves"]).astype(np.int32, copy=False)
    W1 = np.asarray(inputs["W1"], dtype=np.float32)
    b1 = np.asarray(inputs["b1"], dtype=np.float32)
    W2 = np.asarray(inputs["W2"], dtype=np.float32)
    b2 = np.asarray(inputs["b2"], dtype=np.float32)
    W3 = np.asarray(inputs["W3"], dtype=np.float32)
    b3 = np.asarray(inputs["b3"], dtype=np.float32)
    Wl = np.asarray(inputs["Wl"], dtype=np.float32)
    bl = np.asarray(inputs["bl"], dtype=np.float32)

    M1, M2, M3 = _conv_mats(W1, W2, W3)
    WlT = np.concatenate([Wl.T.astype(np.float32), bl[None, :]], axis=0)
    # pack conv matrices into [128, n_tiles*width] partition tiles and all 13
    # per-tile bias columns into one [128, 13] array (single DMA each)
    M2p = np.zeros((128, 7 * F2), np.float32)
    for i, kb in enumerate(range(0, F1, 128)):
        kn = min(128, F1 - kb)
        M2p[:kn, i * F2:(i + 1) * F2] = M2[kb:kb + kn]
    M3p = np.zeros((128, 5 * F3), np.float32)
    for i, kb in enumerate(range(0, F2, 128)):
        kn = min(128, F2 - kb)
        M3p[:kn, i * F3:(i + 1) * F3] = M3[kb:kb + kn]
    b1v = np.repeat(b1, 25).astype(np.float32)
    b2v = np.repeat(b2, 9).astype(np.float32)
    ball = np.zeros((128, 13), np.float32)
    _col = 0
    for vec, width in ((b1v, 128), (b2v, 128), (b3.astype(np.float32), 128)):
        for kb in range(0, len(vec), 128):
            kn = min(128, len(vec) - kb)
            ball[:kn, _col] = vec[kb:kb + kn]
            _col += 1
    assert _col == 13

    # per-row scatter indices, chunked to local_scatter's num_elems limit
    pmi = np.empty((B, 2, K), np.int16)
    for ci, (base, size) in enumerate(SCATTER_CHUNKS):
        inr = (pm >= base) & (pm < base + size)
        pmi[:, ci, :] = np.where(inr, pm - base, -1).astype(np.int16)
    # pack to [core, p, t, 2K]: partition p of row-tile t holds row t*128+p
    pmi = pmi.reshape(NCORES, NRT, 128, 2 * K).transpose(0, 2, 1, 3)
    pmi = np.ascontiguousarray(pmi.reshape(NCORES, 128, NRT * 2 * K))
    # host-built mask for the 4 columns local_scatter can't reach
    mk4 = np.zeros((B, 4), np.float32)
    for i in range(4):
        mk4[:, i] = (pm == MK4_BASE + i).any(axis=1)
    mk4 = mk4.astype(mybir.dt.np(E_DT))
    mk4 = mk4.reshape(NCORES, NRT, 128, 4).transpose(0, 2, 1, 3)
    mk4 = np.ascontiguousarray(mk4.reshape(NCORES, 128, NRT * 4))

    xTall = np.ascontiguousarray(x.T)   # [49, B]

    nc = _get_nc()
    in_maps = []
    for c in range(NCORES):
        sl = slice(c * BC, (c + 1) * BC)
        in_maps.append({
            "xT": np.ascontiguousarray(xTall[:, sl]),
            "m1": M1, "m2": M2p, "m3": M3p, "wlT": WlT,
            "ball": ball,
            "pmi": pmi[c],
            "mk4": mk4[c],
        })

    trace = bool(int(os.environ.get("KERNEL_TRACE", "0")))
    res = run_bass_kernel_spmd(nc, in_maps, list(range(NCORES)), trace=trace)
    _CACHE["last_results"] = res
    _CACHE["last_in_maps"] = in_maps
    out = np.concatenate(
        [np.asarray(res.results[i]["out"]) for i in range(NCORES)], axis=0)
    return out.astype(np.float32)
